# revision 41
# baseline (speedup 1.0000x reference)
"""GCNConv Trainium2 kernel (8 NeuronCores, Bass/Tile).

out = relu( D^{-1/2} (A + I) D^{-1/2} (x W^T + b) )

Distribution: destination nodes (output rows) are sharded across 8 cores.
Edges are partitioned by destination row so the segment-sum is core-local.
x is replicated to every core's HBM; each core gathers the source rows it
needs via the SWDGE dma_gather instruction. The small weight/bias are
replicated.

Device algorithm per core (dest rows R_m, |R_m| = N/8):
  reorder:  agg[n] = sum_{e: dst=n} norm[e] * x[src[e]]      (gather + one-hot matmul)
            out[n] = relu( agg[n] @ W^T + P1[n] * b )        (P1[n] = sum norm over row n)
  where norm/P1 (pure degree-normalization scalars) are computed on host as
  part of the edge partitioning pass.

Self-loops are NOT gathered: each core's own x slab (its dest rows, permuted
into group/slot order, bf16) is bulk-copied and consumed as one diagonal
selection chunk per group, which shrinks the padded gather extent.

Segment-sum on device: each core's destinations are packed into groups of
<=128 (greedy assignment balancing per-bank edge counts); edges land in
gather slots, one per SBUF partition, 128-slot chunks. For each chunk a
selection matrix S[e, d] = norm[e] * (pos[e] == iota_g[d]) is built in bf16
on the vector engine, then PE accumulates aggT += G^T S into the group's
[128,128] PSUM tile ([128,128] matmuls; the bf16 moving operand streams one
row per cycle at any width). A second PE matmul applies W plus the bias
outer-product, ScalarE applies relu into a bf16 output slab stored
batch-wise; the host un-permutes.

Slot layout: groups are processed in gather batches (pairs sharing a
[128, 256] PSUM tile, singles at the tail). Within a (batch, bank) segment each group's edge run has
capacity max-over-cores of its edge count (no per-group 128-ceil), packed
contiguously; segments round to 16 slots (int16 idx wrap granularity).
Chunks therefore straddle group boundaries at build-time-known offsets: a
straddling chunk gets one S-build + matmul per touched group, with the
group's iota slice (values gq*128..gq*128+127) selecting only its edges
(dest values encode gq*128+slot; pads use a sentinel matching no group).
Pad indices gather row 0, so stale gather-pool reads stay finite; the first
_GBUFS batches (one per pool buffer) round segments to full 128-chunks so
every pool byte is written before any stale reuse.

dma_gather uses int16 indices, so the gather source x is addressed in banks
of 32768 rows; one gather call per (batch, bank) with static counts.
"""

import math

import numpy as np

_N_CORES = 8
_P = 128  # partitions / feature dim / dest-group width
_BANK = 32768  # int16-addressable rows per gather bank
_GB = 2  # dest groups per gather batch
_NS = 18  # selection-tile ring depth
_GBUFS = 5  # gather pool ring depth
_SENT = 1000.0  # pad sentinel (matches no iota value)


def _batch_plan(G):
    """Gather-batch sizes: a small first batch shortens the pipeline head;
    single-group tail batches shrink the un-overlapped tail compute."""
    if G <= _GB:
        return [(0, G)]
    plan = []
    g = 0
    while G - g > 3:
        sz = min(_GB, G - g - 3)
        plan.append((g, sz))
        g += sz
    while g < G:
        plan.append((g, 1))
        g += 1
    return plan


def _layout(G, NB, cap, plan):
    """Slot/column layout shared by host prep and program build.

    Returns (run_slot0[G][NB], bb: list per batch of per-bank
    (slot0, length, col0), total_slots, total_cols).
    """
    run_slot0 = np.zeros((G, NB), np.int64)
    bb = []
    slot = 0
    col = 0
    for t, (g0, gsz) in enumerate(plan):
        per_bank = []
        for b in range(NB):
            s0 = slot
            for g in range(g0, g0 + gsz):
                run_slot0[g, b] = slot
                slot += cap[g][b]
            grain = 128 if t < _GBUFS else 16
            ln = -(-(slot - s0) // grain) * grain
            slot = s0 + ln
            per_bank.append((s0, ln, col))
            col += -(-ln // _P)
        bb.append(per_bank)
    return run_slot0, bb, slot, col


_program_cache: dict = {}


# ---------------------------------------------------------------- host prep

def _host_prep(x, W, b, edge_weight, edge_index, n_cores):
    from ml_dtypes import bfloat16

    N, D = x.shape
    assert D == _P
    assert N % n_cores == 0
    nd = N // n_cores  # dest rows per core
    G = math.ceil(nd / _P)  # dest groups per core
    NB = math.ceil(N / _BANK)  # gather banks

    ei = np.asarray(edge_index)
    row = ei[0].astype(np.int64)
    col = ei[1].astype(np.int64)
    w = np.asarray(edge_weight, np.float64)

    # degree normalization (self-loop weight 1 included in the row sums)
    deg = 1.0 + np.bincount(row, weights=w, minlength=N)
    d_inv = 1.0 / np.sqrt(deg)
    norm = d_inv[row] * w * d_inv[col]
    norm_self = d_inv * d_inv
    p1 = (norm_self + np.bincount(row, weights=norm, minlength=N)).astype(np.float32)

    core_e = row // nd
    loc_e = row - core_e * nd
    bank_e = col // _BANK

    # --- balanced dest->group assignment (per core) ---
    # Greedily pack each core's dests into G groups of <=128, balancing the
    # per-bank edge counts so the per-(group,bank) run capacities (maxima
    # over cores) carry minimal padding.
    import heapq

    d_b = np.zeros((NB, N), np.int64)
    for bb_ in range(NB):
        d_b[bb_] = np.bincount(row[bank_e == bb_], minlength=N)
    d_last = d_b[-1].reshape(n_cores, nd)
    d_rest = d_b[:-1].sum(axis=0).reshape(n_cores, nd) if NB > 1 else np.zeros(
        (n_cores, nd), np.int64
    )

    grp_of = np.zeros((n_cores, nd), np.int64)
    slot_of = np.zeros((n_cores, nd), np.int64)
    for m in range(n_cores):
        dl_last = d_last[m]
        dl_rest = d_rest[m]
        cnt = np.zeros(G, np.int64)
        bl = np.zeros(G, np.int64)  # last-bank load
        br = np.zeros(G, np.int64)  # other-banks load
        p1_ids = np.where(dl_last > 0)[0]
        p1_ids = p1_ids[np.lexsort((-dl_rest[p1_ids], -dl_last[p1_ids]))]
        p2_ids = np.where(dl_last == 0)[0]
        p2_ids = p2_ids[np.argsort(-dl_rest[p2_ids], kind="stable")]
        heap = [(0, 0, g) for g in range(G)]
        for dl in p1_ids:
            while True:
                b1v, b0v, g = heapq.heappop(heap)
                if b1v == bl[g] and b0v == br[g] and cnt[g] < _P:
                    break
            grp_of[m, dl] = g
            slot_of[m, dl] = cnt[g]
            cnt[g] += 1
            bl[g] += dl_last[dl]
            br[g] += dl_rest[dl]
            if cnt[g] < _P:
                heapq.heappush(heap, (bl[g], br[g], g))
        heap = [(br[g], g) for g in range(G) if cnt[g] < _P]
        heapq.heapify(heap)
        for dl in p2_ids:
            while True:
                b0v, g = heapq.heappop(heap)
                if b0v == br[g] and cnt[g] < _P:
                    break
            grp_of[m, dl] = g
            slot_of[m, dl] = cnt[g]
            cnt[g] += 1
            br[g] += dl_rest[dl]
            if cnt[g] < _P:
                heapq.heappush(heap, (br[g], g))
    pos_of = grp_of * _P + slot_of  # [M, nd] position in padded output space

    grp_e = grp_of[core_e, loc_e]
    slot_e = slot_of[core_e, loc_e]

    # per-(core, group, bank) edge counts -> run capacities (max over cores)
    gid = (core_e * G + grp_e) * NB + bank_e
    counts = np.bincount(gid, minlength=n_cores * G * NB).reshape(n_cores, G, NB)
    cap = counts.max(axis=0)  # [G, NB]

    plan = _batch_plan(G)
    cap_t = tuple(tuple(int(v) for v in cg) for cg in cap)
    run_slot0, bb, total_slots, C = _layout(G, NB, cap_t, plan)

    # batch index of each group (for dest encoding gq = g - g0)
    g0_of = np.zeros(G, np.int64)
    t_of_g = np.zeros(G, np.int64)
    for t, (g0, gsz) in enumerate(plan):
        g0_of[g0 : g0 + gsz] = g0
        t_of_g[g0 : g0 + gsz] = t

    # edge slot assignment: position within the (core, group, bank) run
    order = np.lexsort((bank_e, grp_e, core_e))
    cs = col[order]
    bs = bank_e[order]
    ns = norm[order]
    core_s = core_e[order]
    grp_s = grp_e[order]
    slot_s = slot_e[order]
    gid_s = (core_s * G + grp_s) * NB + bs
    starts = np.zeros(n_cores * G * NB, np.int64)
    starts[1:] = np.cumsum(counts.reshape(-1))[:-1]
    s = np.arange(len(cs), dtype=np.int64) - starts[gid_s]
    j = run_slot0[grp_s, bs] + s  # global slot

    # global slot -> (column, partition): per (batch, bank) local chunking
    bb_slot0 = np.zeros((len(plan), NB), np.int64)
    bb_col0 = np.zeros((len(plan), NB), np.int64)
    for t in range(len(plan)):
        for b_ in range(NB):
            bb_slot0[t, b_], _, bb_col0[t, b_] = bb[t][b_]
    t_s = t_of_g[grp_s]
    jl = j - bb_slot0[t_s, bs]
    ccol = bb_col0[t_s, bs] + jl // _P
    prow = jl % _P

    dest_arr = np.full((n_cores, _P, C), _SENT, np.float32)
    norm_arr = np.zeros((n_cores, _P, C), np.float32)
    flat = (core_s * _P + prow) * C + ccol
    dest_arr.reshape(-1)[flat] = ((grp_s - g0_of[grp_s]) * _P + slot_s).astype(
        np.float32
    )
    norm_arr.reshape(-1)[flat] = ns.astype(np.float32)

    # int16 gather indices: global slot j -> idx16[j%16, j//16]; pads 0
    idx16 = np.zeros((n_cores, 16, total_slots // 16), np.int16)
    iflat = (core_s * 16 + j % 16) * (total_slots // 16) + j // 16
    idx16.reshape(-1)[iflat] = (cs - bs * _BANK).astype(np.int16)
    idx_tile = np.tile(idx16, (1, 8, 1))  # replicate down 128 partitions

    # self-loop slab: core m's dest rows of x (bf16), permuted to (slot, group)
    GP = G * _P
    xself = np.zeros((n_cores, _P, GP), bfloat16)
    nself = np.zeros((n_cores, _P, G), np.float32)
    p1_arr = np.zeros((n_cores, 1, GP + _P), np.float32)
    x_bf = np.asarray(x, np.float32).astype(bfloat16)
    for m in range(n_cores):
        rows = np.arange(nd, dtype=np.int64)
        g = grp_of[m]
        sl = slot_of[m]
        xs = xself[m].reshape(_P, G, _P)
        xs[sl, g, :] = x_bf[m * nd + rows]
        nself[m][sl, g] = norm_self[m * nd + rows].astype(np.float32)
        p1_arr[m, 0, pos_of[m]] = p1[m * nd + rows]
    iota = np.tile(
        np.arange(_GB * _P, dtype=np.float32), (_P, 1)
    )  # [128, 256]
    iota_bf = np.tile(np.arange(_P, dtype=np.float32), (_P, 1)).astype(bfloat16)
    pidx = np.arange(_P, dtype=np.float32).reshape(_P, 1)
    wT = np.ascontiguousarray(np.asarray(W, np.float32).T)
    bias = np.asarray(b, np.float32).reshape(1, _P).astype(bfloat16)
    x_f32 = np.ascontiguousarray(np.asarray(x, np.float32))

    cfg = (N, nd, G, cap_t, n_cores)
    in_maps = []
    for m in range(n_cores):
        in_maps.append(
            {
                "x": x_f32,
                "idx": idx_tile[m],
                "dest": dest_arr[m],
                "enorm": norm_arr[m],
                "p1": p1_arr[m].astype(bfloat16),
                "xself": xself[m],
                "nself": nself[m],
                "wT": wT,
                "bias": bias,
                "iota": iota,
                "iota_bf": iota_bf,
                "pidx": pidx,
            }
        )
    return cfg, in_maps, pos_of


# ---------------------------------------------------------------- device program

def _build_program(cfg):
    from concourse import bacc, mybir, tile

    N, nd, G, cap, n_cores = cfg
    NB = len(cap[0])
    plan = _batch_plan(G)
    run_slot0, bb, total_slots, C = _layout(G, NB, cap, plan)
    GP = G * _P
    f32 = mybir.dt.float32
    f32r = mybir.dt.float32r
    bf16 = mybir.dt.bfloat16
    i16 = mybir.dt.int16

    nc = bacc.Bacc(
        "TRN2",
        target_bir_lowering=False,
        debug=False,
        enable_asserts=False,
        num_devices=n_cores,
    )
    x_d = nc.dram_tensor("x", [N, _P], f32r, kind="ExternalInput").ap()
    idx_d = nc.dram_tensor(
        "idx", [_P, total_slots // 16], i16, kind="ExternalInput"
    ).ap()
    dest_d = nc.dram_tensor("dest", [_P, C], f32, kind="ExternalInput").ap()
    norm_d = nc.dram_tensor("enorm", [_P, C], f32, kind="ExternalInput").ap()
    p1_d = nc.dram_tensor("p1", [1, GP + _P], bf16, kind="ExternalInput").ap()
    xself_d = nc.dram_tensor("xself", [_P, GP], bf16, kind="ExternalInput").ap()
    nself_d = nc.dram_tensor("nself", [_P, G], f32, kind="ExternalInput").ap()
    wT_d = nc.dram_tensor("wT", [_P, _P], f32r, kind="ExternalInput").ap()
    b_d = nc.dram_tensor("bias", [1, _P], bf16, kind="ExternalInput").ap()
    iota_d = nc.dram_tensor("iota", [_P, _GB * _P], f32, kind="ExternalInput").ap()
    iotab_d = nc.dram_tensor("iota_bf", [_P, _P], bf16, kind="ExternalInput").ap()
    pidx_d = nc.dram_tensor("pidx", [_P, 1], f32, kind="ExternalInput").ap()
    out_d = nc.dram_tensor("outT", [_P, GP], bf16, kind="ExternalOutput").ap()
    W2 = _GB * _P  # pair width (two groups share one PSUM tile)

    with tile.TileContext(nc) as tc:
        with (
            tc.tile_pool(name="const", bufs=1) as cpool,
            tc.tile_pool(name="gather", bufs=_GBUFS) as gpool,
            tc.tile_pool(name="agg", bufs=3) as apool,
            tc.tile_pool(name="ps1", bufs=6, space="PSUM") as ps1pool,
            tc.tile_pool(name="ps2", bufs=2, space="PSUM") as ps2pool,
        ):
            idx_t = cpool.tile([_P, total_slots // 16], i16)
            dest_t = cpool.tile([_P, C], f32)
            norm_t = cpool.tile([_P, C], f32)
            iota_t = cpool.tile([_P, _GB * _P], f32)
            iotab_t = cpool.tile([_P, _P], bf16)
            pidx_t = cpool.tile([_P, 1], f32)
            xself_t = cpool.tile([_P, GP], bf16)
            nself_t = cpool.tile([_P, G], f32)
            # first-batch slices go first so the gather + selection pipeline
            # starts as early as possible; everything else loads behind them
            s1 = bb[0][NB - 1][0] + bb[0][NB - 1][1]  # first-batch slot extent
            c1 = bb[0][NB - 1][2] + bb[0][NB - 1][1] // _P  # col extent
            nc.gpsimd.dma_start(out=idx_t[:, : s1 // 16], in_=idx_d[:, : s1 // 16])
            nc.scalar.dma_start(out=iota_t[:], in_=iota_d)
            nc.scalar.dma_start(out=iotab_t[:], in_=iotab_d)
            nc.scalar.dma_start(out=pidx_t[:], in_=pidx_d)
            nc.scalar.dma_start(out=nself_t[:], in_=nself_d)
            nc.sync.dma_start(
                out=xself_t[:, : plan[0][1] * _P], in_=xself_d[:, : plan[0][1] * _P]
            )
            nc.scalar.dma_start(out=dest_t[:, :c1], in_=dest_d[:, :c1])
            nc.scalar.dma_start(out=norm_t[:, :c1], in_=norm_d[:, :c1])
            nc.sync.dma_start(out=idx_t[:, s1 // 16 :], in_=idx_d[:, s1 // 16 :])
            nc.sync.dma_start(out=dest_t[:, c1:], in_=dest_d[:, c1:])
            nc.sync.dma_start(out=norm_t[:, c1:], in_=norm_d[:, c1:])
            nc.sync.dma_start(
                out=xself_t[:, plan[0][1] * _P :], in_=xself_d[:, plan[0][1] * _P :]
            )
            wT_t = cpool.tile([_P, _P], f32r)
            nc.sync.dma_start(out=wT_t[:], in_=wT_d)
            b_t = cpool.tile([1, _P], bf16)
            nc.sync.dma_start(out=b_t[:], in_=b_d)
            p1_t = cpool.tile([1, GP + _P], bf16)
            nc.sync.dma_start(out=p1_t[:], in_=p1_d)
            out_t = cpool.tile([_P, GP + _P], bf16)

            # selection-tile rings, [128, 256] f32r per pair batch. Pure
            # chunks of pair-position q only ever write cols q*128..q*128+127
            # (the other half stays zero from the initial memset); straddling
            # chunks use a dedicated ring whose tiles are always fully
            # written (both halves per use).
            s_half = [[], []]
            for h in range(2):
                zlo = (1 - h) * _P
                for i in range(_NS):
                    st = cpool.tile([_P, W2], f32r, tag=f"sh{h}_{i}", name=f"sh{h}_{i}")
                    nc.vector.memset(st[:, zlo : zlo + _P].bitcast(f32), 0.0)
                    s_half[h].append(st)
            s_both = []
            for i in range(4):
                st = cpool.tile([_P, W2], f32r, tag=f"sb{i}", name=f"sb{i}")
                s_both.append(st)
            s_self = [[], []]
            for h in range(2):
                zlo = (1 - h) * _P
                for i in range(4):
                    st = cpool.tile(
                        [_P, W2], bf16, tag=f"ss{h}_{i}", name=f"ss{h}_{i}"
                    )
                    nc.vector.memset(st[:, zlo : zlo + _P], 0.0)
                    s_self[h].append(st)
            ss_rr = [0, 0]
            s_rr = [0, 0, 0]

            def next_s(h):
                if h < 2:
                    st = s_half[h][s_rr[h]]
                    s_rr[h] = (s_rr[h] + 1) % _NS
                else:
                    st = s_both[s_rr[2]]
                    s_rr[2] = (s_rr[2] + 1) % 4
                return st

            max_nch = [
                max(-(-bb[t][b_][1] // _P) for t in range(len(plan)))
                for b_ in range(NB)
            ]
            stored = 0
            for t, (g0, gsz) in enumerate(plan):
                g1 = g0 + gsz
                gts = []
                nmm = gsz  # self chunks
                for b in range(NB):
                    s0, ln, c0 = bb[t][b]
                    nmm += -(-ln // _P)
                gts = []
                for b in range(NB):
                    s0, ln, c0 = bb[t][b]
                    gt = gpool.tile(
                        [_P, max_nch[b] * _P], f32r, tag=f"g{b}", name=f"gt{b}"
                    )
                    lo = b * _BANK
                    hi = min(N, lo + _BANK)
                    nc.gpsimd.dma_gather(
                        out_ap=gt[:, : -(-ln // _P) * _P].rearrange(
                            "p (c e) -> p c e", e=_P
                        ),
                        in_ap=x_d[lo:hi, :],
                        idxs_ap=idx_t[:, s0 // 16 : (s0 + ln) // 16],
                        num_idxs=ln,
                        num_idxs_reg=ln,
                        elem_size=_P,
                        single_packet=False,
                    )
                    gts.append(gt)
                ps1 = ps1pool.tile([_P, W2], f32, tag="ps1", name="ps1")
                imm = 0
                # self-loop chunks first: PE work available before the
                # batch's gather lands
                for g in range(g0, g1):
                    gq = g - g0
                    S = s_self[gq][ss_rr[gq]]
                    ss_rr[gq] = (ss_rr[gq] + 1) % 4
                    nc.vector.tensor_scalar(
                        out=S[:, gq * _P : (gq + 1) * _P],
                        in0=iotab_t[:],
                        scalar1=pidx_t[:, 0:1],
                        scalar2=nself_t[:, g : g + 1],
                        op0=mybir.AluOpType.is_equal,
                        op1=mybir.AluOpType.mult,
                    )
                    nc.tensor.matmul(
                        out=ps1[:],
                        lhsT=xself_t[:, g * _P : (g + 1) * _P],
                        rhs=S[:],
                        start=(imm == 0),
                        stop=(imm == nmm - 1),
                    )
                    imm += 1
                for b in range(NB):
                    s0, ln, c0 = bb[t][b]
                    for cl in range(-(-ln // _P)):
                        c = c0 + cl
                        chunk_lo = s0 + cl * _P
                        chunk_hi = chunk_lo + _P
                        touched = [
                            g
                            for g in range(g0, g1)
                            if not (
                                int(run_slot0[g, b]) + cap[g][b] <= chunk_lo
                                or int(run_slot0[g, b]) >= chunk_hi
                            )
                        ]
                        if len(touched) == 1:
                            h = touched[0] - g0
                        else:
                            h = 2
                        S = next_s(h)
                        for g in touched:
                            gq = g - g0
                            nc.vector.tensor_scalar(
                                out=S[:, gq * _P : (gq + 1) * _P],
                                in0=iota_t[:, gq * _P : (gq + 1) * _P],
                                scalar1=dest_t[:, c : c + 1],
                                scalar2=norm_t[:, c : c + 1],
                                op0=mybir.AluOpType.is_equal,
                                op1=mybir.AluOpType.mult,
                            )
                        nc.tensor.matmul(
                            out=ps1[:],
                            lhsT=gts[b][:, cl * _P : (cl + 1) * _P],
                            rhs=S[:],
                            start=(imm == 0),
                            stop=(imm == nmm - 1),
                        )
                        imm += 1
                # pair epilogue: W + bias outer product, relu, store
                aggT = apool.tile([_P, W2], f32r, tag="a", name="aggT")
                nc.scalar.copy(out=aggT[:], in_=ps1[:])
                ps2 = ps2pool.tile([_P, W2], f32, tag="ps2", name="ps2")
                nc.tensor.matmul(
                    out=ps2[:],
                    lhsT=b_t[:],
                    rhs=p1_t[:, g0 * _P : g0 * _P + W2],
                    start=True,
                    stop=False,
                )
                nc.tensor.matmul(
                    out=ps2[:], lhsT=wT_t[:], rhs=aggT[:], start=False, stop=True
                )
                nc.scalar.activation(
                    out=out_t[:, g0 * _P : g0 * _P + W2],
                    in_=ps2[:],
                    func=mybir.ActivationFunctionType.Relu,
                )
                if gsz >= 2 or g1 == G:
                    nc.sync.dma_start(
                        out=out_d[:, stored * _P : g1 * _P],
                        in_=out_t[:, stored * _P : g1 * _P],
                    )
                    stored = g1

    nc.compile()
    return nc


def _get_program(cfg):
    if cfg not in _program_cache:
        _program_cache[cfg] = _build_program(cfg)
    return _program_cache[cfg]


# ---------------------------------------------------------------- entry points

def run(inputs: dict, trace: bool = False, n_cores: int = _N_CORES):
    """Run the kernel; returns (full_output, BassKernelResults)."""
    from concourse import bass_utils

    cfg, in_maps, pos_of = _host_prep(
        inputs["x"],
        inputs["W"],
        inputs["b"],
        inputs["edge_weight"],
        inputs["edge_index"],
        n_cores,
    )
    nc = _get_program(cfg)
    try:
        res = bass_utils.run_bass_kernel_spmd(
            nc, in_maps, core_ids=list(range(n_cores)), trace=trace
        )
    except Exception:
        # the axon-tunneled device occasionally reports a transient
        # NRT_EXEC_UNIT_UNRECOVERABLE right after a crashed/heavy prior run;
        # reconnect the backend and retry once before giving up
        import time as _time

        import jax as _jax

        _time.sleep(5.0)
        try:
            _jax.clear_backends()
        except Exception:
            pass
        res = bass_utils.run_bass_kernel_spmd(
            nc, in_maps, core_ids=list(range(n_cores)), trace=trace
        )
    N, nd = cfg[0], cfg[1]
    out = np.empty((N, _P), np.float32)
    for m in range(n_cores):
        slab = np.asarray(res.results[m]["outT"]).astype(np.float32).T  # [GP, 128]
        out[m * nd : (m + 1) * nd, :] = slab[pos_of[m]]
    return out, res


def kernel(**inputs) -> np.ndarray:
    out, _ = run(inputs, trace=False)
    return out
